# revision 22
# baseline (speedup 1.0000x reference)
"""Evo2 attention (B=2, S=2048, HID=2048, NH=16, HD=128) on 8 trn2 NeuronCores.

Sharding: core c handles batch b=c//4 and heads 4*(c%4)..4*(c%4)+3.
Megatron-style: q/k/v projections column-parallel, o_proj row-parallel with the
4-way partial sum done on host during unshard.

Per-core kernel layout (everything transposed so no on-chip transposes needed):
  hsT [hid, tok] -> qT,kT [hd, tok] (RoPE fused into PSUM eviction, rotate-half
  basis obtained by de-interleaving W rows on host), v [tok, hd].
  scoresT[k, q] = kT_blk vs qT matmul; softmax over k (= partitions) with a
  fixed shift instead of a max; denominators via ones-vector PE reduction and
  a K=1 matmul broadcast; PV gives attnT [hd, q]; o_projT partial [o, q].

All DRAM inputs are host-pre-tiled so each DMA lands partition-contiguous
(16-32KB runs per partition) - descriptor-count is what bounds the DMA rings.
"""
import os
import sys
import numpy as np

for _p in ("/opt/trn_rl_repo",):
    if os.path.isdir(_p) and _p not in sys.path:
        sys.path.insert(0, _p)

B, S, HID, NH = 2, 2048, 2048, 16
HD = HID // NH            # 128
HPC = 4                   # heads per core
NCORES = 8
BASE = 10000.0
SCALE = 1.0 / np.sqrt(HD).astype(np.float32)
SHIFT = 25.0              # fixed softmax shift (replaces per-row max)
NEG_INF_THRESH = -1e8

_PROGRAM_CACHE = {}


def _rope_tables():
    """cos/±sin tables [HD, S] in the de-interleaved (rotate-half) basis.

    Reference pairs dims (2m, 2m+1) with angle theta_m(s) = s * inv_freq[f(m)],
    f(m) = 2m for m<32 else 2m-64 (from emb[:, ::2] of concat([freqs, freqs])).
    After de-interleave perm [0,2,..126,1,3,..127]: new dim m<64 is old 2m,
    new dim 64+m is old 2m+1.
      out[m]    = x[m] cos_m - x[64+m] sin_m
      out[64+m] = x[m] sin_m + x[64+m] cos_m
    """
    inv_freq = BASE ** (-np.arange(0, HD, 2, dtype=np.float64) / HD)  # [64]
    m = np.arange(64)
    fmap = np.where(m < 32, 2 * m, 2 * m - 64)
    t = np.arange(S, dtype=np.float64)
    theta = t[None, :] * inv_freq[fmap][:, None]          # [64, S]
    cos = np.cos(theta)
    sin = np.sin(theta)
    cosT = np.concatenate([cos, cos], axis=0).astype(np.float32)      # [128, S]
    # row d holds the factor applied to SOURCE half d (dest = other half):
    # src lo -> dst hi uses +sin; src hi -> dst lo uses -sin
    ssinT = np.concatenate([sin, -sin], axis=0).astype(np.float32)    # [128, S]
    return cosT, ssinT


def _mask_plan(mask2d):
    """Classify [128k x 512q] blocks of mask^T. Returns (plan, tiles).

    plan[qc] = list of (kb, mask_tile_idx_or_None); fully-masked blocks skipped.
    tiles: deduped f32 [128, 512] mask^T blocks prescaled by sqrt(HD).
    """
    maskT = np.ascontiguousarray(mask2d.T)  # [k, q]
    plan = []
    tiles = []
    seen = {}
    for qc in range(S // 512):
        row = []
        for kb in range(S // 128):
            sub = maskT[kb * 128:(kb + 1) * 128, qc * 512:(qc + 1) * 512]
            if (sub <= NEG_INF_THRESH).all():
                continue
            if (sub == 0.0).all():
                row.append((kb, None))
                continue
            pre = np.ascontiguousarray(sub * np.float32(np.sqrt(HD)))
            key = pre.tobytes()
            idx = seen.get(key)
            if idx is None:
                idx = len(tiles)
                seen[key] = idx
                tiles.append(pre)
            row.append((kb, idx))
        plan.append(row)
    return plan, tiles


def _build_program(plan, nm, preload_masks):
    import contextlib
    import concourse.bacc as bacc
    import concourse.tile as tile
    from concourse import mybir

    f32 = mybir.dt.float32
    f32r = mybir.dt.float32r
    nc = bacc.Bacc(None, target_bir_lowering=False)

    # host-pre-tiled inputs: last axis group per partition is contiguous
    hs_d = nc.dram_tensor("hs_t", [4, 128, 16, 512], f32r, kind="ExternalInput")
    wq_d = nc.dram_tensor("wq_t", [128, 16, 4, 128], f32r, kind="ExternalInput")
    wk_d = nc.dram_tensor("wk_t", [128, 16, 4, 128], f32r, kind="ExternalInput")
    wv_d = nc.dram_tensor("wv_t", [128, 16, 512], f32r, kind="ExternalInput")
    wo_d = nc.dram_tensor("wo_t", [128, 4, 16, 128], f32r, kind="ExternalInput")
    cos_d = nc.dram_tensor("cosT", [128, S], f32, kind="ExternalInput")
    sin_d = nc.dram_tensor("ssinT", [128, S], f32, kind="ExternalInput")
    if nm:
        mask_d = nc.dram_tensor("maskt", [nm, 128, 512], f32, kind="ExternalInput")
    o_d = nc.dram_tensor("oT_t", [4, 16, 128, 512], f32, kind="ExternalOutput")

    Exp = mybir.ActivationFunctionType.Exp

    with tile.TileContext(nc) as tc:
        with contextlib.ExitStack() as perm:
            kt_pool = perm.enter_context(tc.tile_pool(name="kt", bufs=4))
            qa_pool = perm.enter_context(tc.tile_pool(name="qa", bufs=17))
            cst = perm.enter_context(tc.tile_pool(name="cst", bufs=1))
            tmp_pool = perm.enter_context(tc.tile_pool(name="tmp", bufs=2))

            ones_st = cst.tile([128, 1], f32, tag="o1")
            nc.vector.memset(ones_st, 1.0)
            ones_r = cst.tile([128, 1], f32r, tag="o2")
            nc.vector.tensor_copy(out=ones_r, in_=ones_st)
            ones1_st = cst.tile([1, 128], f32, tag="o3")
            nc.vector.memset(ones1_st, 1.0)
            ones1_r = cst.tile([1, 128], f32r, tag="o4")
            nc.vector.tensor_copy(out=ones1_r, in_=ones1_st)
            shiftb = cst.tile([128, 1], f32, tag="sh")
            nc.vector.memset(shiftb, -SHIFT)

            kt = [kt_pool.tile([128, S], f32r, tag="kt", name=f"kt{i}") for i in range(HPC)]
            v_tiles = [None] * 16
            q_tiles = {}
            attn_tiles = {}

            def rope_evict(ps, dst, cos_sb, sin_sb):
                # ACT (idle in phase A) evicts PSUM; DVE then runs SBUF-only
                s = tmp_pool.tile([128, 512], f32, tag="ropes")
                nc.scalar.copy(out=s[:], in_=ps[:])
                nc.vector.tensor_mul(dst[0:64, :], s[64:128, :], sin_sb[64:128, :])
                nc.vector.tensor_mul(dst[64:128, :], s[0:64, :], sin_sb[0:64, :])
                t = tmp_pool.tile([128, 512], f32, tag="ropetmp")
                nc.vector.tensor_mul(t[:], s[:], cos_sb[:, :])
                nc.vector.tensor_add(dst[:, :], dst[:, :], t[:])

            # ---- Phase A: projections. hs pool is shared between the q/k
            # pass and the v pass so the v-pass prefetch overlaps the q/k tail.
            with contextlib.ExitStack() as actx:
                hs_pool = actx.enter_context(tc.tile_pool(name="hs", bufs=5))
                tabp = actx.enter_context(tc.tile_pool(name="tab", bufs=2))
                psA = actx.enter_context(tc.tile_pool(name="psA", bufs=3, space="PSUM"))

                def load_hs(c, tagsfx):
                    qts = []
                    for hf in range(4):
                        ht = hs_pool.tile([128, 4, 512], f32r, tag="hs",
                                          name=f"hs{tagsfx}{c}_{hf}")
                        nc.sync.dma_start(
                            out=ht, in_=hs_d[c, :, hf * 4:(hf + 1) * 4, :])
                        qts.append(ht)
                    return qts

                with contextlib.ExitStack() as wctx:
                    w_pool = wctx.enter_context(tc.tile_pool(name="w", bufs=1))
                    # first chunk's hs first so the first matmul isn't gated
                    # on 8MB of weights; weights stream in kc-quarters.
                    halves = load_hs(0, "")
                    wq_all = w_pool.tile([128, 16, 4, 128], f32r, tag="wq", name="wqall")
                    wk_all = w_pool.tile([128, 16, 4, 128], f32r, tag="wk", name="wkall")
                    for qtr in range(4):
                        nc.sync.dma_start(out=wq_all[:, qtr * 4:(qtr + 1) * 4, :, :],
                                          in_=wq_d[:, qtr * 4:(qtr + 1) * 4, :, :])
                    for qtr in range(4):
                        nc.sync.dma_start(out=wk_all[:, qtr * 4:(qtr + 1) * 4, :, :],
                                          in_=wk_d[:, qtr * 4:(qtr + 1) * 4, :, :])
                    for c in range(4):
                        if c > 0:
                            halves = load_hs(c, "")
                        cos_sb = tabp.tile([128, 512], f32, tag="cos")
                        nc.gpsimd.dma_start(out=cos_sb, in_=cos_d[:, c * 512:(c + 1) * 512])
                        sin_sb = tabp.tile([128, 512], f32, tag="sin")
                        nc.gpsimd.dma_start(out=sin_sb, in_=sin_d[:, c * 512:(c + 1) * 512])
                        for pass_i, w_all in ((0, wq_all), (1, wk_all)):
                            for dblk in range(4):
                                ps = psA.tile([128, 512], f32, tag="qk")
                                for kc in range(16):
                                    nc.tensor.matmul(
                                        ps[:], w_all[:, kc, dblk, :],
                                        halves[kc // 4][:, kc % 4, :],
                                        start=(kc == 0), stop=(kc == 15))
                                if pass_i == 0:
                                    q = qa_pool.tile([128, 512], f32r, tag="qa")
                                    rope_evict(ps, q, cos_sb, sin_sb)
                                    q_tiles[(dblk, c)] = q
                                else:
                                    rope_evict(ps[:, :], kt[dblk][:, c * 512:(c + 1) * 512],
                                               cos_sb, sin_sb)

                # ---- v projection (pools claim the space w_pool freed) ----
                # right side: lets the actx "left" stack pop in LIFO order
                v_pool = perm.enter_context(tc.tile_pool(name="v", bufs=16, side="right"))
                with contextlib.ExitStack() as wctx:
                    wv_pool = wctx.enter_context(tc.tile_pool(name="wv", bufs=1))
                    psAv = wctx.enter_context(tc.tile_pool(name="psAv", bufs=4, space="PSUM"))

                    wv_all = wv_pool.tile([128, 16, 512], f32r, tag="wv", name="wvall")
                    for qtr in range(4):
                        nc.sync.dma_start(out=wv_all[:, qtr * 4:(qtr + 1) * 4, :],
                                          in_=wv_d[:, qtr * 4:(qtr + 1) * 4, :])
                    for c in range(4):
                        halves = load_hs(c, "v")
                        psv = [psAv.tile([128, 512], f32, tag="v", name=f"psv{i}")
                               for i in range(4)]
                        for kc in range(16):
                            for vblk in range(4):
                                nc.tensor.matmul(
                                    psv[vblk][:],
                                    halves[kc // 4][:, kc % 4, vblk * 128:(vblk + 1) * 128],
                                    wv_all[:, kc, :], start=(kc == 0), stop=(kc == 15))
                        for vblk in range(4):
                            vt = v_pool.tile([128, 512], f32r, tag="v", name=f"v{c}_{vblk}")
                            nc.scalar.copy(out=vt[:], in_=psv[vblk][:])
                            v_tiles[c * 4 + vblk] = vt

            # ---------------- Phase B + C ------------------------------------
            with contextlib.ExitStack() as bctx:
                wo_pool = bctx.enter_context(tc.tile_pool(name="wo", bufs=1))
                probs_pool = bctx.enter_context(tc.tile_pool(name="pr", bufs=8))
                smx_pool = bctx.enter_context(tc.tile_pool(name="sm", bufs=3))
                den_pool = bctx.enter_context(tc.tile_pool(name="dn", bufs=2))
                rcb_pool = bctx.enter_context(tc.tile_pool(name="rcb", bufs=2))
                outb_pool = bctx.enter_context(tc.tile_pool(name="ob", bufs=3))
                psB_s = bctx.enter_context(tc.tile_pool(name="psBs", bufs=2, space="PSUM"))
                psB_a = bctx.enter_context(tc.tile_pool(name="psBa", bufs=3, space="PSUM"))
                psB_d = bctx.enter_context(tc.tile_pool(name="psBd", bufs=1, space="PSUM"))
                psC = bctx.enter_context(tc.tile_pool(name="psC", bufs=2, space="PSUM"))
                mask_sb = None
                if nm and preload_masks:
                    mp = bctx.enter_context(tc.tile_pool(name="mk", bufs=nm))
                    mask_sb = []
                    for i in range(nm):
                        mt = mp.tile([128, 512], f32, tag="mk", name=f"mk{i}")
                        nc.gpsimd.dma_start(out=mt, in_=mask_d[i, :, :])
                        mask_sb.append(mt)
                elif nm:
                    mp = bctx.enter_context(tc.tile_pool(name="mk", bufs=8))

                wo_all = wo_pool.tile([128, 4, 16, 128], f32r, tag="wo", name="woall")
                nc.sync.dma_start(out=wo_all, in_=wo_d[:, :, :, :])

                def emit_tail(state):
                    h, qc, ps_att, den_sb = state
                    ps_b = psB_s.tile([128, 512], f32, tag="s", name="psbc")
                    nc.tensor.matmul(ps_b[:], ones1_r[:], den_sb[:], start=True, stop=True)
                    rcb = rcb_pool.tile([128, 512], f32, tag="rcb")
                    nc.vector.reciprocal(out=rcb[:], in_=ps_b[:])
                    at = qa_pool.tile([128, 512], f32r, tag="qa")
                    nc.vector.tensor_mul(at[:], ps_att[:], rcb[:])
                    attn_tiles[(h, qc)] = at

                def emit_c_chunk(qc, iblks):
                    for iblk in iblks:
                        ps_o = psC.tile([128, 512], f32, tag="o")
                        for jc in range(4):
                            nc.tensor.matmul(ps_o[:], wo_all[:, jc, iblk, :],
                                             attn_tiles[(jc, qc)][:],
                                             start=(jc == 0), stop=(jc == 3))
                        ob = outb_pool.tile([128, 512], f32, tag="ob")
                        if iblk % 2:
                            nc.vector.tensor_copy(out=ob[:], in_=ps_o[:])
                        else:
                            nc.scalar.copy(out=ob[:], in_=ps_o[:])
                        nc.sync.dma_start(out=o_d[qc, iblk, :, :], in_=ob[:])

                tail_state = None
                for qc in range(4):
                    kbs = plan[qc]
                    nkb = len(kbs)
                    for h in range(HPC):
                        ps_att = psB_a.tile([128, 512], f32, tag="att")
                        ps_den = psB_d.tile([1, 512], f32, tag="d")
                        qtile = q_tiles[(h, qc)]
                        for j, (kb, mi) in enumerate(kbs):
                            ps_s = psB_s.tile([128, 512], f32, tag="s")
                            nc.tensor.matmul(ps_s[:], kt[h][:, kb * 128:(kb + 1) * 128],
                                             qtile[:], start=True, stop=True)
                            if mi is not None:
                                if preload_masks:
                                    msb = mask_sb[mi]
                                else:
                                    msb = mp.tile([128, 512], f32, tag="mk", name=f"mks{mi}")
                                    nc.gpsimd.dma_start(out=msb, in_=mask_d[mi, :, :])
                                # psum-read sbuf-write: in-place psum add would
                                # halve DVE rate (single psum port, read+write)
                                sm = smx_pool.tile([128, 512], f32, tag="sm")
                                nc.vector.tensor_add(sm[:], ps_s[:], msb[:])
                                exp_src = sm
                            else:
                                exp_src = ps_s
                            pr = probs_pool.tile([128, 512], f32r, tag="pr")
                            nc.scalar.activation(pr[:], exp_src[:], Exp,
                                                 bias=shiftb[:], scale=float(SCALE))
                            nc.tensor.matmul(ps_den[:], ones_r[:], pr[:],
                                             start=(j == 0), stop=(j == nkb - 1))
                            nc.tensor.matmul(ps_att[:],
                                             v_tiles[kb][:, h * 128:(h + 1) * 128],
                                             pr[:], start=(j == 0), stop=(j == nkb - 1))
                        # evict den now (frees the [1,512] psum bank for next h);
                        # the rest of the tail is deferred one head for pipelining
                        den_sb = den_pool.tile([1, 512], f32r, tag="dn")
                        nc.scalar.copy(out=den_sb[:], in_=ps_den[:])
                        if tail_state is not None:
                            emit_tail(tail_state)
                        tail_state = (h, qc, ps_att, den_sb)
                        # spread previous qc's o-projection as PE stall filler
                        if qc > 0:
                            emit_c_chunk(qc - 1, range(4 * h, 4 * h + 4))
                    if qc > 0:
                        for hh in range(HPC):
                            del attn_tiles[(hh, qc - 1)]
                emit_tail(tail_state)
                emit_c_chunk(3, range(16))
                for hh in range(HPC):
                    del attn_tiles[(hh, 3)]

    nc.compile()
    return nc


LAST_EXEC_NS = None


def kernel(hidden_states, Wq, Wk, Wv, Wo, attention_mask):
    global LAST_EXEC_NS
    from concourse.bass_utils import run_bass_kernel_spmd

    hidden_states = np.asarray(hidden_states, dtype=np.float32)
    Wq = np.asarray(Wq, dtype=np.float32)
    Wk = np.asarray(Wk, dtype=np.float32)
    Wv = np.asarray(Wv, dtype=np.float32)
    Wo = np.asarray(Wo, dtype=np.float32)
    attention_mask = np.asarray(attention_mask, dtype=np.float32)

    cosT, ssinT = _rope_tables()
    plan, mtiles = _mask_plan(attention_mask[0])
    nm = len(mtiles)
    preload = nm <= 24
    maskt = np.stack(mtiles) if nm else None

    plan_key = (tuple(tuple(r) for r in plan), nm, preload)
    nc = _PROGRAM_CACHE.get(plan_key)
    if nc is None:
        nc = _build_program(plan, nm, preload)
        _PROGRAM_CACHE[plan_key] = nc

    perm = np.concatenate([np.arange(0, HD, 2), np.arange(1, HD, 2)])
    Wq4 = Wq.reshape(NH, HD, HID)[:, perm, :]
    Wk4 = Wk.reshape(NH, HD, HID)[:, perm, :]
    Wv4 = Wv.reshape(NH, HD, HID)

    # [4, 128, 16, 512] per-partition-contiguous hs tiling, per batch
    hs_tl = [np.ascontiguousarray(
        hidden_states[b].reshape(4, 512, 16, 128).transpose(0, 3, 2, 1))
        for b in range(B)]

    def tile_qk(mT):   # [HID, 512] -> [128, 16, 4, 128]
        return np.ascontiguousarray(
            mT.reshape(16, 128, 4, 128).transpose(1, 0, 2, 3))

    in_maps = []
    for c in range(NCORES):
        b, hg = divmod(c, HPC)
        heads = slice(hg * HPC, (hg + 1) * HPC)
        wqT = Wq4[heads].reshape(512, HID).T          # [HID, 512]
        wkT = Wk4[heads].reshape(512, HID).T
        wvT = Wv4[heads].reshape(512, HID).T          # [HID, 512]
        woT = Wo[:, hg * 512:(hg + 1) * 512].T        # [512, HID]
        m = {
            "hs_t": hs_tl[b],
            "wq_t": tile_qk(wqT),
            "wk_t": tile_qk(wkT),
            "wv_t": np.ascontiguousarray(
                wvT.reshape(16, 128, 512).transpose(1, 0, 2)),
            "wo_t": np.ascontiguousarray(
                woT.reshape(4, 128, 16, 128).transpose(1, 0, 2, 3)),
            "cosT": cosT,
            "ssinT": ssinT,
        }
        if nm:
            m["maskt"] = maskt
        in_maps.append(m)

    trace = bool(os.environ.get("CC_BASS_TRACE"))
    res = run_bass_kernel_spmd(nc, in_maps, core_ids=list(range(NCORES)), trace=trace)
    LAST_EXEC_NS = res.exec_time_ns

    out = np.empty((B, S, S), dtype=np.float32)
    for b in range(B):
        acc = res.results[b * HPC]["oT_t"].astype(np.float32)
        for hg in range(1, HPC):
            acc = acc + res.results[b * HPC + hg]["oT_t"]
        # [qc, iblk, p, t] -> [iblk*128+p, qc*512+t] = oT_full, out = oT_full.T
        o_full = acc.transpose(1, 2, 0, 3).reshape(S, S)
        out[b] = o_full.T
    return out


# revision 23
# speedup vs baseline: 1.0346x; 1.0346x over previous
"""Evo2 attention (B=2, S=2048, HID=2048, NH=16, HD=128) on 8 trn2 NeuronCores.

Sharding: core c handles batch b=c//4 and heads 4*(c%4)..4*(c%4)+3.
Megatron-style: q/k/v projections column-parallel, o_proj row-parallel with the
4-way partial sum done on host during unshard.

Per-core kernel layout (everything transposed so no on-chip transposes needed):
  hsT [hid, tok] -> qT,kT [hd, tok] (RoPE fused into PSUM eviction, rotate-half
  basis obtained by de-interleaving W rows on host), v [tok, hd].
  scoresT[k, q] = kT_blk vs qT matmul; softmax over k (= partitions) with a
  fixed shift instead of a max; denominators via ones-vector PE reduction and
  a K=1 matmul broadcast; PV gives attnT [hd, q]; o_projT partial [o, q].

All DRAM inputs are host-pre-tiled so each DMA lands partition-contiguous
(16-32KB runs per partition) - descriptor-count is what bounds the DMA rings.
"""
import os
import sys
import numpy as np

for _p in ("/opt/trn_rl_repo",):
    if os.path.isdir(_p) and _p not in sys.path:
        sys.path.insert(0, _p)

B, S, HID, NH = 2, 2048, 2048, 16
HD = HID // NH            # 128
HPC = 4                   # heads per core
NCORES = 8
BASE = 10000.0
SCALE = 1.0 / np.sqrt(HD).astype(np.float32)
SHIFT = 25.0              # fixed softmax shift (replaces per-row max)
NEG_INF_THRESH = -1e8

_PROGRAM_CACHE = {}


def _rope_tables():
    """cos/±sin tables [HD, S] in the de-interleaved (rotate-half) basis.

    Reference pairs dims (2m, 2m+1) with angle theta_m(s) = s * inv_freq[f(m)],
    f(m) = 2m for m<32 else 2m-64 (from emb[:, ::2] of concat([freqs, freqs])).
    After de-interleave perm [0,2,..126,1,3,..127]: new dim m<64 is old 2m,
    new dim 64+m is old 2m+1.
      out[m]    = x[m] cos_m - x[64+m] sin_m
      out[64+m] = x[m] sin_m + x[64+m] cos_m
    """
    inv_freq = BASE ** (-np.arange(0, HD, 2, dtype=np.float64) / HD)  # [64]
    m = np.arange(64)
    fmap = np.where(m < 32, 2 * m, 2 * m - 64)
    t = np.arange(S, dtype=np.float64)
    theta = t[None, :] * inv_freq[fmap][:, None]          # [64, S]
    cos = np.cos(theta)
    sin = np.sin(theta)
    cosT = np.concatenate([cos, cos], axis=0).astype(np.float32)      # [128, S]
    # row d holds the factor applied to SOURCE half d (dest = other half):
    # src lo -> dst hi uses +sin; src hi -> dst lo uses -sin
    ssinT = np.concatenate([sin, -sin], axis=0).astype(np.float32)    # [128, S]
    return cosT, ssinT


def _mask_plan(mask2d):
    """Classify [128k x 512q] blocks of mask^T. Returns (plan, tiles).

    plan[qc] = list of (kb, mask_tile_idx_or_None); fully-masked blocks skipped.
    tiles: deduped f32 [128, 512] mask^T blocks prescaled by sqrt(HD).
    """
    maskT = np.ascontiguousarray(mask2d.T)  # [k, q]
    plan = []
    tiles = []
    seen = {}
    for qc in range(S // 512):
        row = []
        for kb in range(S // 128):
            sub = maskT[kb * 128:(kb + 1) * 128, qc * 512:(qc + 1) * 512]
            if (sub <= NEG_INF_THRESH).all():
                continue
            if (sub == 0.0).all():
                row.append((kb, None))
                continue
            pre = np.ascontiguousarray(sub * np.float32(np.sqrt(HD)))
            key = pre.tobytes()
            idx = seen.get(key)
            if idx is None:
                idx = len(tiles)
                seen[key] = idx
                tiles.append(pre)
            row.append((kb, idx))
        plan.append(row)
    return plan, tiles


def _build_program(plan, nm, preload_masks):
    import contextlib
    import concourse.bacc as bacc
    import concourse.tile as tile
    from concourse import mybir

    f32 = mybir.dt.float32
    f32r = mybir.dt.float32r
    nc = bacc.Bacc(None, target_bir_lowering=False)

    # host-pre-tiled inputs: last axis group per partition is contiguous
    hs_d = nc.dram_tensor("hs_t", [4, 128, 16, 512], f32r, kind="ExternalInput")
    wq_d = nc.dram_tensor("wq_t", [128, 16, 4, 128], f32r, kind="ExternalInput")
    wk_d = nc.dram_tensor("wk_t", [128, 16, 4, 128], f32r, kind="ExternalInput")
    wv_d = nc.dram_tensor("wv_t", [128, 16, 512], f32r, kind="ExternalInput")
    wo_d = nc.dram_tensor("wo_t", [128, 4, 16, 128], f32r, kind="ExternalInput")
    cos_d = nc.dram_tensor("cosT", [128, S], f32, kind="ExternalInput")
    sin_d = nc.dram_tensor("ssinT", [128, S], f32, kind="ExternalInput")
    if nm:
        mask_d = nc.dram_tensor("maskt", [nm, 128, 512], f32, kind="ExternalInput")
    o_d = nc.dram_tensor("oT_t", [4, 16, 128, 512], f32, kind="ExternalOutput")

    Exp = mybir.ActivationFunctionType.Exp

    with tile.TileContext(nc) as tc:
        with contextlib.ExitStack() as perm:
            kt_pool = perm.enter_context(tc.tile_pool(name="kt", bufs=4))
            qa_pool = perm.enter_context(tc.tile_pool(name="qa", bufs=17))
            cst = perm.enter_context(tc.tile_pool(name="cst", bufs=1))
            tmp_pool = perm.enter_context(tc.tile_pool(name="tmp", bufs=2))

            ones_st = cst.tile([128, 1], f32, tag="o1")
            nc.vector.memset(ones_st, 1.0)
            ones_r = cst.tile([128, 1], f32r, tag="o2")
            nc.vector.tensor_copy(out=ones_r, in_=ones_st)
            onesq_st = cst.tile([128, 128], f32, tag="o3")
            nc.vector.memset(onesq_st, 1.0)
            onesq_r = cst.tile([128, 128], f32r, tag="o4")
            nc.vector.tensor_copy(out=onesq_r, in_=onesq_st)
            shiftb = cst.tile([128, 1], f32, tag="sh")
            nc.vector.memset(shiftb, -SHIFT)

            kt = [kt_pool.tile([128, S], f32r, tag="kt", name=f"kt{i}") for i in range(HPC)]
            v_tiles = [None] * 16
            q_tiles = {}
            attn_tiles = {}

            def rope_evict(ps, dst, cos_sb, sin_sb):
                nc.vector.tensor_mul(dst[0:64, :], ps[64:128, :], sin_sb[64:128, :])
                nc.vector.tensor_mul(dst[64:128, :], ps[0:64, :], sin_sb[0:64, :])
                t = tmp_pool.tile([128, 512], f32, tag="ropetmp")
                nc.vector.tensor_mul(t[:], ps[:], cos_sb[:, :])
                nc.vector.tensor_add(dst[:, :], dst[:, :], t[:])

            # ---- Phase A: projections. hs pool is shared between the q/k
            # pass and the v pass so the v-pass prefetch overlaps the q/k tail.
            with contextlib.ExitStack() as actx:
                hs_pool = actx.enter_context(tc.tile_pool(name="hs", bufs=5))
                tabp = actx.enter_context(tc.tile_pool(name="tab", bufs=2))
                psA = actx.enter_context(tc.tile_pool(name="psA", bufs=3, space="PSUM"))

                def load_hs(c, tagsfx):
                    qts = []
                    for hf in range(4):
                        ht = hs_pool.tile([128, 4, 512], f32r, tag="hs",
                                          name=f"hs{tagsfx}{c}_{hf}")
                        nc.sync.dma_start(
                            out=ht, in_=hs_d[c, :, hf * 4:(hf + 1) * 4, :])
                        qts.append(ht)
                    return qts

                with contextlib.ExitStack() as wctx:
                    w_pool = wctx.enter_context(tc.tile_pool(name="w", bufs=1))
                    # first chunk's hs first so the first matmul isn't gated
                    # on 8MB of weights; weights stream in kc-quarters.
                    halves = load_hs(0, "")
                    wq_all = w_pool.tile([128, 16, 4, 128], f32r, tag="wq", name="wqall")
                    wk_all = w_pool.tile([128, 16, 4, 128], f32r, tag="wk", name="wkall")
                    for qtr in range(4):
                        nc.sync.dma_start(out=wq_all[:, qtr * 4:(qtr + 1) * 4, :, :],
                                          in_=wq_d[:, qtr * 4:(qtr + 1) * 4, :, :])
                    for qtr in range(4):
                        nc.sync.dma_start(out=wk_all[:, qtr * 4:(qtr + 1) * 4, :, :],
                                          in_=wk_d[:, qtr * 4:(qtr + 1) * 4, :, :])
                    for c in range(4):
                        if c > 0:
                            halves = load_hs(c, "")
                        cos_sb = tabp.tile([128, 512], f32, tag="cos")
                        nc.gpsimd.dma_start(out=cos_sb, in_=cos_d[:, c * 512:(c + 1) * 512])
                        sin_sb = tabp.tile([128, 512], f32, tag="sin")
                        nc.gpsimd.dma_start(out=sin_sb, in_=sin_d[:, c * 512:(c + 1) * 512])
                        for pass_i, w_all in ((0, wq_all), (1, wk_all)):
                            for dblk in range(4):
                                ps = psA.tile([128, 512], f32, tag="qk")
                                for kc in range(16):
                                    nc.tensor.matmul(
                                        ps[:], w_all[:, kc, dblk, :],
                                        halves[kc // 4][:, kc % 4, :],
                                        start=(kc == 0), stop=(kc == 15))
                                if pass_i == 0:
                                    q = qa_pool.tile([128, 512], f32r, tag="qa")
                                    rope_evict(ps, q, cos_sb, sin_sb)
                                    q_tiles[(dblk, c)] = q
                                else:
                                    rope_evict(ps[:, :], kt[dblk][:, c * 512:(c + 1) * 512],
                                               cos_sb, sin_sb)

                # ---- v projection (pools claim the space w_pool freed) ----
                # right side: lets the actx "left" stack pop in LIFO order
                v_pool = perm.enter_context(tc.tile_pool(name="v", bufs=16, side="right"))
                with contextlib.ExitStack() as wctx:
                    wv_pool = wctx.enter_context(tc.tile_pool(name="wv", bufs=1))
                    psAv = wctx.enter_context(tc.tile_pool(name="psAv", bufs=4, space="PSUM"))

                    wv_all = wv_pool.tile([128, 16, 512], f32r, tag="wv", name="wvall")
                    for qtr in range(4):
                        nc.sync.dma_start(out=wv_all[:, qtr * 4:(qtr + 1) * 4, :],
                                          in_=wv_d[:, qtr * 4:(qtr + 1) * 4, :])
                    for c in range(4):
                        halves = load_hs(c, "v")
                        psv = [psAv.tile([128, 512], f32, tag="v", name=f"psv{i}")
                               for i in range(4)]
                        for kc in range(16):
                            for vblk in range(4):
                                nc.tensor.matmul(
                                    psv[vblk][:],
                                    halves[kc // 4][:, kc % 4, vblk * 128:(vblk + 1) * 128],
                                    wv_all[:, kc, :], start=(kc == 0), stop=(kc == 15))
                        for vblk in range(4):
                            vt = v_pool.tile([128, 512], f32r, tag="v", name=f"v{c}_{vblk}")
                            nc.scalar.copy(out=vt[:], in_=psv[vblk][:])
                            v_tiles[c * 4 + vblk] = vt

            # ---------------- Phase B + C ------------------------------------
            with contextlib.ExitStack() as bctx:
                wo_pool = bctx.enter_context(tc.tile_pool(name="wo", bufs=1))
                probs_pool = bctx.enter_context(tc.tile_pool(name="pr", bufs=8))
                smx_pool = bctx.enter_context(tc.tile_pool(name="sm", bufs=3))
                den_pool = bctx.enter_context(tc.tile_pool(name="dn", bufs=2))
                rcb_pool = bctx.enter_context(tc.tile_pool(name="rcb", bufs=2))
                outb_pool = bctx.enter_context(tc.tile_pool(name="ob", bufs=3))
                psB_s = bctx.enter_context(tc.tile_pool(name="psBs", bufs=2, space="PSUM"))
                psB_a = bctx.enter_context(tc.tile_pool(name="psBa", bufs=3, space="PSUM"))
                psB_d = bctx.enter_context(tc.tile_pool(name="psBd", bufs=1, space="PSUM"))
                psC = bctx.enter_context(tc.tile_pool(name="psC", bufs=2, space="PSUM"))
                mask_sb = None
                if nm and preload_masks:
                    mp = bctx.enter_context(tc.tile_pool(name="mk", bufs=nm))
                    mask_sb = []
                    for i in range(nm):
                        mt = mp.tile([128, 512], f32, tag="mk", name=f"mk{i}")
                        nc.gpsimd.dma_start(out=mt, in_=mask_d[i, :, :])
                        mask_sb.append(mt)
                elif nm:
                    mp = bctx.enter_context(tc.tile_pool(name="mk", bufs=8))

                wo_all = wo_pool.tile([128, 4, 16, 128], f32r, tag="wo", name="woall")
                nc.sync.dma_start(out=wo_all, in_=wo_d[:, :, :, :])

                def emit_tail(state):
                    h, qc, ps_att, den_sb = state
                    rcb = rcb_pool.tile([128, 512], f32, tag="rcb")
                    nc.vector.reciprocal(out=rcb[:], in_=den_sb[:])
                    at = qa_pool.tile([128, 512], f32r, tag="qa")
                    nc.vector.tensor_mul(at[:], ps_att[:], rcb[:])
                    attn_tiles[(h, qc)] = at

                def emit_c_chunk(qc, iblks):
                    for iblk in iblks:
                        ps_o = psC.tile([128, 512], f32, tag="o")
                        for jc in range(4):
                            nc.tensor.matmul(ps_o[:], wo_all[:, jc, iblk, :],
                                             attn_tiles[(jc, qc)][:],
                                             start=(jc == 0), stop=(jc == 3))
                        ob = outb_pool.tile([128, 512], f32, tag="ob")
                        if iblk % 2:
                            nc.vector.tensor_copy(out=ob[:], in_=ps_o[:])
                        else:
                            nc.scalar.copy(out=ob[:], in_=ps_o[:])
                        nc.sync.dma_start(out=o_d[qc, iblk, :, :], in_=ob[:])

                tail_state = None
                for qc in range(4):
                    kbs = plan[qc]
                    nkb = len(kbs)
                    for h in range(HPC):
                        ps_att = psB_a.tile([128, 512], f32, tag="att")
                        ps_den = psB_d.tile([128, 512], f32, tag="d")
                        qtile = q_tiles[(h, qc)]
                        for j, (kb, mi) in enumerate(kbs):
                            ps_s = psB_s.tile([128, 512], f32, tag="s")
                            nc.tensor.matmul(ps_s[:], kt[h][:, kb * 128:(kb + 1) * 128],
                                             qtile[:], start=True, stop=True)
                            if mi is not None:
                                if preload_masks:
                                    msb = mask_sb[mi]
                                else:
                                    msb = mp.tile([128, 512], f32, tag="mk", name=f"mks{mi}")
                                    nc.gpsimd.dma_start(out=msb, in_=mask_d[mi, :, :])
                                # psum-read sbuf-write: in-place psum add would
                                # halve DVE rate (single psum port, read+write)
                                sm = smx_pool.tile([128, 512], f32, tag="sm")
                                nc.vector.tensor_add(sm[:], ps_s[:], msb[:])
                                exp_src = sm
                            else:
                                exp_src = ps_s
                            pr = probs_pool.tile([128, 512], f32r, tag="pr")
                            nc.scalar.activation(pr[:], exp_src[:], Exp,
                                                 bias=shiftb[:], scale=float(SCALE))
                            nc.tensor.matmul(ps_den[:], onesq_r[:], pr[:],
                                             start=(j == 0), stop=(j == nkb - 1))
                            nc.tensor.matmul(ps_att[:],
                                             v_tiles[kb][:, h * 128:(h + 1) * 128],
                                             pr[:], start=(j == 0), stop=(j == nkb - 1))
                        # evict den now (frees the den psum bank for next h);
                        # the rest of the tail is deferred one head for pipelining
                        den_sb = den_pool.tile([128, 512], f32, tag="dn")
                        nc.scalar.copy(out=den_sb[:], in_=ps_den[:])
                        if tail_state is not None:
                            emit_tail(tail_state)
                        tail_state = (h, qc, ps_att, den_sb)
                        # spread previous qc's o-projection as PE stall filler
                        if qc > 0:
                            emit_c_chunk(qc - 1, range(4 * h, 4 * h + 4))
                    if qc > 0:
                        for hh in range(HPC):
                            del attn_tiles[(hh, qc - 1)]
                emit_tail(tail_state)
                emit_c_chunk(3, range(16))
                for hh in range(HPC):
                    del attn_tiles[(hh, 3)]

    nc.compile()
    return nc


LAST_EXEC_NS = None


def kernel(hidden_states, Wq, Wk, Wv, Wo, attention_mask):
    global LAST_EXEC_NS
    from concourse.bass_utils import run_bass_kernel_spmd

    hidden_states = np.asarray(hidden_states, dtype=np.float32)
    Wq = np.asarray(Wq, dtype=np.float32)
    Wk = np.asarray(Wk, dtype=np.float32)
    Wv = np.asarray(Wv, dtype=np.float32)
    Wo = np.asarray(Wo, dtype=np.float32)
    attention_mask = np.asarray(attention_mask, dtype=np.float32)

    cosT, ssinT = _rope_tables()
    plan, mtiles = _mask_plan(attention_mask[0])
    nm = len(mtiles)
    preload = nm <= 24
    maskt = np.stack(mtiles) if nm else None

    plan_key = (tuple(tuple(r) for r in plan), nm, preload)
    nc = _PROGRAM_CACHE.get(plan_key)
    if nc is None:
        nc = _build_program(plan, nm, preload)
        _PROGRAM_CACHE[plan_key] = nc

    perm = np.concatenate([np.arange(0, HD, 2), np.arange(1, HD, 2)])
    Wq4 = Wq.reshape(NH, HD, HID)[:, perm, :]
    Wk4 = Wk.reshape(NH, HD, HID)[:, perm, :]
    Wv4 = Wv.reshape(NH, HD, HID)

    # [4, 128, 16, 512] per-partition-contiguous hs tiling, per batch
    hs_tl = [np.ascontiguousarray(
        hidden_states[b].reshape(4, 512, 16, 128).transpose(0, 3, 2, 1))
        for b in range(B)]

    def tile_qk(mT):   # [HID, 512] -> [128, 16, 4, 128]
        return np.ascontiguousarray(
            mT.reshape(16, 128, 4, 128).transpose(1, 0, 2, 3))

    in_maps = []
    for c in range(NCORES):
        b, hg = divmod(c, HPC)
        heads = slice(hg * HPC, (hg + 1) * HPC)
        wqT = Wq4[heads].reshape(512, HID).T          # [HID, 512]
        wkT = Wk4[heads].reshape(512, HID).T
        wvT = Wv4[heads].reshape(512, HID).T          # [HID, 512]
        woT = Wo[:, hg * 512:(hg + 1) * 512].T        # [512, HID]
        m = {
            "hs_t": hs_tl[b],
            "wq_t": tile_qk(wqT),
            "wk_t": tile_qk(wkT),
            "wv_t": np.ascontiguousarray(
                wvT.reshape(16, 128, 512).transpose(1, 0, 2)),
            "wo_t": np.ascontiguousarray(
                woT.reshape(4, 128, 16, 128).transpose(1, 0, 2, 3)),
            "cosT": cosT,
            "ssinT": ssinT,
        }
        if nm:
            m["maskt"] = maskt
        in_maps.append(m)

    trace = bool(os.environ.get("CC_BASS_TRACE"))
    res = run_bass_kernel_spmd(nc, in_maps, core_ids=list(range(NCORES)), trace=trace)
    LAST_EXEC_NS = res.exec_time_ns

    out = np.empty((B, S, S), dtype=np.float32)
    for b in range(B):
        acc = res.results[b * HPC]["oT_t"].astype(np.float32)
        for hg in range(1, HPC):
            acc = acc + res.results[b * HPC + hg]["oT_t"]
        # [qc, iblk, p, t] -> [iblk*128+p, qc*512+t] = oT_full, out = oT_full.T
        o_full = acc.transpose(1, 2, 0, 3).reshape(S, S)
        out[b] = o_full.T
    return out


# revision 24
# speedup vs baseline: 1.0702x; 1.0344x over previous
"""Evo2 attention (B=2, S=2048, HID=2048, NH=16, HD=128) on 8 trn2 NeuronCores.

Sharding: core c handles batch b=c//4 and heads 4*(c%4)..4*(c%4)+3.
Megatron-style: q/k/v projections column-parallel, o_proj row-parallel with the
4-way partial sum done on host during unshard.

Per-core kernel layout (everything transposed so no on-chip transposes needed):
  hsT [hid, tok] -> qT,kT [hd, tok] (RoPE fused into PSUM eviction, rotate-half
  basis obtained by de-interleaving W rows on host), v [tok, hd].
  scoresT[k, q] = kT_blk vs qT matmul; softmax over k (= partitions) with a
  fixed shift instead of a max; denominators via ones-vector PE reduction and
  a K=1 matmul broadcast; PV gives attnT [hd, q]; o_projT partial [o, q].

All DRAM inputs are host-pre-tiled so each DMA lands partition-contiguous
(16-32KB runs per partition) - descriptor-count is what bounds the DMA rings.
"""
import os
import sys
import numpy as np

for _p in ("/opt/trn_rl_repo",):
    if os.path.isdir(_p) and _p not in sys.path:
        sys.path.insert(0, _p)

B, S, HID, NH = 2, 2048, 2048, 16
HD = HID // NH            # 128
HPC = 4                   # heads per core
NCORES = 8
BASE = 10000.0
SCALE = 1.0 / np.sqrt(HD).astype(np.float32)
SHIFT = 25.0              # fixed softmax shift (replaces per-row max)
NEG_INF_THRESH = -1e8

_PROGRAM_CACHE = {}


def _rope_tables():
    """cos/±sin tables [HD, S] in the de-interleaved (rotate-half) basis.

    Reference pairs dims (2m, 2m+1) with angle theta_m(s) = s * inv_freq[f(m)],
    f(m) = 2m for m<32 else 2m-64 (from emb[:, ::2] of concat([freqs, freqs])).
    After de-interleave perm [0,2,..126,1,3,..127]: new dim m<64 is old 2m,
    new dim 64+m is old 2m+1.
      out[m]    = x[m] cos_m - x[64+m] sin_m
      out[64+m] = x[m] sin_m + x[64+m] cos_m
    """
    inv_freq = BASE ** (-np.arange(0, HD, 2, dtype=np.float64) / HD)  # [64]
    m = np.arange(64)
    fmap = np.where(m < 32, 2 * m, 2 * m - 64)
    t = np.arange(S, dtype=np.float64)
    theta = t[None, :] * inv_freq[fmap][:, None]          # [64, S]
    cos = np.cos(theta)
    sin = np.sin(theta)
    cosT = np.concatenate([cos, cos], axis=0).astype(np.float32)      # [128, S]
    # row d holds the factor applied to SOURCE half d (dest = other half):
    # src lo -> dst hi uses +sin; src hi -> dst lo uses -sin
    ssinT = np.concatenate([sin, -sin], axis=0).astype(np.float32)    # [128, S]
    return cosT, ssinT


def _mask_plan(mask2d):
    """Classify [128k x 512q] blocks of mask^T. Returns (plan, tiles).

    plan[qc] = list of (kb, mask_tile_idx_or_None); fully-masked blocks skipped.
    tiles: deduped f32 [128, 512] mask^T blocks prescaled by sqrt(HD).
    """
    maskT = np.ascontiguousarray(mask2d.T)  # [k, q]
    plan = []
    tiles = []
    seen = {}
    for qc in range(S // 512):
        row = []
        for kb in range(S // 128):
            sub = maskT[kb * 128:(kb + 1) * 128, qc * 512:(qc + 1) * 512]
            if (sub <= NEG_INF_THRESH).all():
                continue
            if (sub == 0.0).all():
                row.append((kb, None))
                continue
            pre = np.ascontiguousarray(sub * np.float32(np.sqrt(HD)))
            key = pre.tobytes()
            idx = seen.get(key)
            if idx is None:
                idx = len(tiles)
                seen[key] = idx
                tiles.append(pre)
            row.append((kb, idx))
        plan.append(row)
    return plan, tiles


def _build_program(plan, nm, preload_masks):
    import contextlib
    import concourse.bacc as bacc
    import concourse.tile as tile
    from concourse import mybir

    f32 = mybir.dt.float32
    f32r = mybir.dt.float32r
    nc = bacc.Bacc(None, target_bir_lowering=False)

    # host-pre-tiled inputs: last axis group per partition is contiguous
    hs_d = nc.dram_tensor("hs_t", [4, 128, 16, 512], f32r, kind="ExternalInput")
    wq_d = nc.dram_tensor("wq_t", [128, 16, 4, 128], f32r, kind="ExternalInput")
    wk_d = nc.dram_tensor("wk_t", [128, 16, 4, 128], f32r, kind="ExternalInput")
    wv_d = nc.dram_tensor("wv_t", [128, 16, 512], f32r, kind="ExternalInput")
    wo_d = nc.dram_tensor("wo_t", [128, 4, 16, 128], f32r, kind="ExternalInput")
    cos_d = nc.dram_tensor("cosT", [128, S], f32, kind="ExternalInput")
    sin_d = nc.dram_tensor("ssinT", [128, S], f32, kind="ExternalInput")
    if nm:
        mask_d = nc.dram_tensor("maskt", [nm, 128, 512], f32, kind="ExternalInput")
    o_d = nc.dram_tensor("oT_t", [4, 16, 128, 512], f32, kind="ExternalOutput")

    Exp = mybir.ActivationFunctionType.Exp

    with tile.TileContext(nc) as tc:
        with contextlib.ExitStack() as perm:
            kt_pool = perm.enter_context(tc.tile_pool(name="kt", bufs=4))
            qa_pool = perm.enter_context(tc.tile_pool(name="qa", bufs=17))
            cst = perm.enter_context(tc.tile_pool(name="cst", bufs=1))
            tmp_pool = perm.enter_context(tc.tile_pool(name="tmp", bufs=2))

            ones_st = cst.tile([128, 1], f32, tag="o1")
            nc.vector.memset(ones_st, 1.0)
            ones_r = cst.tile([128, 1], f32r, tag="o2")
            nc.vector.tensor_copy(out=ones_r, in_=ones_st)
            onesq_st = cst.tile([128, 128], f32, tag="o3")
            nc.vector.memset(onesq_st, 1.0)
            onesq_r = cst.tile([128, 128], f32r, tag="o4")
            nc.vector.tensor_copy(out=onesq_r, in_=onesq_st)
            shiftb = cst.tile([128, 1], f32, tag="sh")
            nc.vector.memset(shiftb, -SHIFT)

            kt = [kt_pool.tile([128, S], f32r, tag="kt", name=f"kt{i}") for i in range(HPC)]
            v_tiles = [None] * 16
            q_tiles = {}
            attn_tiles = {}

            def rope_evict(ps, dst, cos_sb, sin_sb):
                nc.vector.tensor_mul(dst[0:64, :], ps[64:128, :], sin_sb[64:128, :])
                nc.vector.tensor_mul(dst[64:128, :], ps[0:64, :], sin_sb[0:64, :])
                t = tmp_pool.tile([128, 512], f32, tag="ropetmp")
                nc.vector.tensor_mul(t[:], ps[:], cos_sb[:, :])
                nc.vector.tensor_add(dst[:, :], dst[:, :], t[:])

            # ---- Phase A: projections. hs pool is shared between the q/k
            # pass and the v pass so the v-pass prefetch overlaps the q/k tail.
            with contextlib.ExitStack() as actx:
                hs_pool = actx.enter_context(tc.tile_pool(name="hs", bufs=6))
                tabp = actx.enter_context(tc.tile_pool(name="tab", bufs=2))
                psA = actx.enter_context(tc.tile_pool(name="psA", bufs=3, space="PSUM"))

                def load_hs(c, tagsfx):
                    qts = []
                    for hf in range(4):
                        ht = hs_pool.tile([128, 4, 512], f32r, tag="hs",
                                          name=f"hs{tagsfx}{c}_{hf}")
                        nc.sync.dma_start(
                            out=ht, in_=hs_d[c, :, hf * 4:(hf + 1) * 4, :])
                        qts.append(ht)
                    return qts

                with contextlib.ExitStack() as wctx:
                    w_pool = wctx.enter_context(tc.tile_pool(name="w", bufs=1))
                    # first chunk's hs first so the first matmul isn't gated
                    # on 8MB of weights; weights stream in kc-quarters.
                    halves = load_hs(0, "")
                    wq_all = w_pool.tile([128, 16, 4, 128], f32r, tag="wq", name="wqall")
                    wk_all = w_pool.tile([128, 16, 4, 128], f32r, tag="wk", name="wkall")
                    for qtr in range(4):
                        nc.sync.dma_start(out=wq_all[:, qtr * 4:(qtr + 1) * 4, :, :],
                                          in_=wq_d[:, qtr * 4:(qtr + 1) * 4, :, :])
                    for qtr in range(4):
                        nc.sync.dma_start(out=wk_all[:, qtr * 4:(qtr + 1) * 4, :, :],
                                          in_=wk_d[:, qtr * 4:(qtr + 1) * 4, :, :])
                    for c in range(4):
                        if c > 0:
                            halves = load_hs(c, "")
                        cos_sb = tabp.tile([128, 512], f32, tag="cos")
                        nc.gpsimd.dma_start(out=cos_sb, in_=cos_d[:, c * 512:(c + 1) * 512])
                        sin_sb = tabp.tile([128, 512], f32, tag="sin")
                        nc.gpsimd.dma_start(out=sin_sb, in_=sin_d[:, c * 512:(c + 1) * 512])
                        for pass_i, w_all in ((0, wq_all), (1, wk_all)):
                            for dblk in range(4):
                                ps = psA.tile([128, 512], f32, tag="qk")
                                for kc in range(16):
                                    nc.tensor.matmul(
                                        ps[:], w_all[:, kc, dblk, :],
                                        halves[kc // 4][:, kc % 4, :],
                                        start=(kc == 0), stop=(kc == 15))
                                if pass_i == 0:
                                    q = qa_pool.tile([128, 512], f32r, tag="qa")
                                    rope_evict(ps, q, cos_sb, sin_sb)
                                    q_tiles[(dblk, c)] = q
                                else:
                                    rope_evict(ps[:, :], kt[dblk][:, c * 512:(c + 1) * 512],
                                               cos_sb, sin_sb)

                # ---- v projection (pools claim the space w_pool freed) ----
                # right side: lets the actx "left" stack pop in LIFO order
                v_pool = perm.enter_context(tc.tile_pool(name="v", bufs=16, side="right"))
                with contextlib.ExitStack() as wctx:
                    wv_pool = wctx.enter_context(tc.tile_pool(name="wv", bufs=1))
                    psAv = wctx.enter_context(tc.tile_pool(name="psAv", bufs=4, space="PSUM"))

                    wv_all = wv_pool.tile([128, 16, 512], f32r, tag="wv", name="wvall")
                    for qtr in range(4):
                        nc.sync.dma_start(out=wv_all[:, qtr * 4:(qtr + 1) * 4, :],
                                          in_=wv_d[:, qtr * 4:(qtr + 1) * 4, :])
                    for c in range(4):
                        halves = load_hs(c, "v")
                        psv = [psAv.tile([128, 512], f32, tag="v", name=f"psv{i}")
                               for i in range(4)]
                        for kc in range(16):
                            for vblk in range(4):
                                nc.tensor.matmul(
                                    psv[vblk][:],
                                    halves[kc // 4][:, kc % 4, vblk * 128:(vblk + 1) * 128],
                                    wv_all[:, kc, :], start=(kc == 0), stop=(kc == 15))
                        for vblk in range(4):
                            vt = v_pool.tile([128, 512], f32r, tag="v", name=f"v{c}_{vblk}")
                            nc.scalar.copy(out=vt[:], in_=psv[vblk][:])
                            v_tiles[c * 4 + vblk] = vt

            # ---------------- Phase B + C ------------------------------------
            with contextlib.ExitStack() as bctx:
                wo_pool = bctx.enter_context(tc.tile_pool(name="wo", bufs=1))
                probs_pool = bctx.enter_context(tc.tile_pool(name="pr", bufs=8))
                smx_pool = bctx.enter_context(tc.tile_pool(name="sm", bufs=3))
                den_pool = bctx.enter_context(tc.tile_pool(name="dn", bufs=2))
                rcb_pool = bctx.enter_context(tc.tile_pool(name="rcb", bufs=2))
                outb_pool = bctx.enter_context(tc.tile_pool(name="ob", bufs=3))
                psB_s = bctx.enter_context(tc.tile_pool(name="psBs", bufs=3, space="PSUM"))
                psB_a = bctx.enter_context(tc.tile_pool(name="psBa", bufs=2, space="PSUM"))
                psB_d = bctx.enter_context(tc.tile_pool(name="psBd", bufs=1, space="PSUM"))
                psC = bctx.enter_context(tc.tile_pool(name="psC", bufs=2, space="PSUM"))
                mask_sb = None
                if nm and preload_masks:
                    mp = bctx.enter_context(tc.tile_pool(name="mk", bufs=nm))
                    mask_sb = []
                    for i in range(nm):
                        mt = mp.tile([128, 512], f32, tag="mk", name=f"mk{i}")
                        nc.gpsimd.dma_start(out=mt, in_=mask_d[i, :, :])
                        mask_sb.append(mt)
                elif nm:
                    mp = bctx.enter_context(tc.tile_pool(name="mk", bufs=8))

                wo_all = wo_pool.tile([128, 4, 16, 128], f32r, tag="wo", name="woall")
                nc.sync.dma_start(out=wo_all, in_=wo_d[:, :, :, :])

                def emit_tail(state):
                    h, qc, ps_att, den_sb = state
                    rcb = rcb_pool.tile([128, 512], f32, tag="rcb")
                    nc.vector.reciprocal(out=rcb[:], in_=den_sb[:])
                    at = qa_pool.tile([128, 512], f32r, tag="qa")
                    nc.vector.tensor_mul(at[:], ps_att[:], rcb[:])
                    attn_tiles[(h, qc)] = at

                def emit_c_chunk(qc, iblks):
                    for iblk in iblks:
                        ps_o = psC.tile([128, 512], f32, tag="o")
                        for jc in range(4):
                            nc.tensor.matmul(ps_o[:], wo_all[:, jc, iblk, :],
                                             attn_tiles[(jc, qc)][:],
                                             start=(jc == 0), stop=(jc == 3))
                        ob = outb_pool.tile([128, 512], f32, tag="ob")
                        if iblk % 8 < 5:
                            nc.vector.tensor_copy(out=ob[:], in_=ps_o[:])
                        else:
                            nc.scalar.copy(out=ob[:], in_=ps_o[:])
                        nc.sync.dma_start(out=o_d[qc, iblk, :, :], in_=ob[:])

                tail_state = None
                for qc in range(4):
                    kbs = plan[qc]
                    nkb = len(kbs)
                    for h in range(HPC):
                        ps_att = psB_a.tile([128, 512], f32, tag="att")
                        ps_den = psB_d.tile([128, 512], f32, tag="d")
                        qtile = q_tiles[(h, qc)]
                        for j, (kb, mi) in enumerate(kbs):
                            ps_s = psB_s.tile([128, 512], f32, tag="s")
                            nc.tensor.matmul(ps_s[:], kt[h][:, kb * 128:(kb + 1) * 128],
                                             qtile[:], start=True, stop=True)
                            if mi is not None:
                                if preload_masks:
                                    msb = mask_sb[mi]
                                else:
                                    msb = mp.tile([128, 512], f32, tag="mk", name=f"mks{mi}")
                                    nc.gpsimd.dma_start(out=msb, in_=mask_d[mi, :, :])
                                # psum-read sbuf-write: in-place psum add would
                                # halve DVE rate (single psum port, read+write)
                                sm = smx_pool.tile([128, 512], f32, tag="sm")
                                nc.vector.tensor_add(sm[:], ps_s[:], msb[:])
                                exp_src = sm
                            else:
                                exp_src = ps_s
                            pr = probs_pool.tile([128, 512], f32r, tag="pr")
                            nc.scalar.activation(pr[:], exp_src[:], Exp,
                                                 bias=shiftb[:], scale=float(SCALE))
                            nc.tensor.matmul(ps_den[:], onesq_r[:], pr[:],
                                             start=(j == 0), stop=(j == nkb - 1))
                            nc.tensor.matmul(ps_att[:],
                                             v_tiles[kb][:, h * 128:(h + 1) * 128],
                                             pr[:], start=(j == 0), stop=(j == nkb - 1))
                        # evict den now (frees the den psum bank for next h);
                        # the rest of the tail is deferred one head for pipelining
                        den_sb = den_pool.tile([128, 512], f32, tag="dn")
                        nc.scalar.copy(out=den_sb[:], in_=ps_den[:])
                        if tail_state is not None:
                            emit_tail(tail_state)
                        tail_state = (h, qc, ps_att, den_sb)
                        # spread previous qc's o-projection as PE stall filler
                        if qc > 0:
                            emit_c_chunk(qc - 1, range(4 * h, 4 * h + 4))
                    if qc > 0:
                        for hh in range(HPC):
                            del attn_tiles[(hh, qc - 1)]
                emit_tail(tail_state)
                emit_c_chunk(3, range(16))
                for hh in range(HPC):
                    del attn_tiles[(hh, 3)]

    nc.compile()
    return nc


LAST_EXEC_NS = None


def kernel(hidden_states, Wq, Wk, Wv, Wo, attention_mask):
    global LAST_EXEC_NS
    from concourse.bass_utils import run_bass_kernel_spmd

    hidden_states = np.asarray(hidden_states, dtype=np.float32)
    Wq = np.asarray(Wq, dtype=np.float32)
    Wk = np.asarray(Wk, dtype=np.float32)
    Wv = np.asarray(Wv, dtype=np.float32)
    Wo = np.asarray(Wo, dtype=np.float32)
    attention_mask = np.asarray(attention_mask, dtype=np.float32)

    cosT, ssinT = _rope_tables()
    plan, mtiles = _mask_plan(attention_mask[0])
    nm = len(mtiles)
    preload = nm <= 24
    maskt = np.stack(mtiles) if nm else None

    plan_key = (tuple(tuple(r) for r in plan), nm, preload)
    nc = _PROGRAM_CACHE.get(plan_key)
    if nc is None:
        nc = _build_program(plan, nm, preload)
        _PROGRAM_CACHE[plan_key] = nc

    perm = np.concatenate([np.arange(0, HD, 2), np.arange(1, HD, 2)])
    Wq4 = Wq.reshape(NH, HD, HID)[:, perm, :]
    Wk4 = Wk.reshape(NH, HD, HID)[:, perm, :]
    Wv4 = Wv.reshape(NH, HD, HID)

    # [4, 128, 16, 512] per-partition-contiguous hs tiling, per batch
    hs_tl = [np.ascontiguousarray(
        hidden_states[b].reshape(4, 512, 16, 128).transpose(0, 3, 2, 1))
        for b in range(B)]

    def tile_qk(mT):   # [HID, 512] -> [128, 16, 4, 128]
        return np.ascontiguousarray(
            mT.reshape(16, 128, 4, 128).transpose(1, 0, 2, 3))

    in_maps = []
    for c in range(NCORES):
        b, hg = divmod(c, HPC)
        heads = slice(hg * HPC, (hg + 1) * HPC)
        wqT = Wq4[heads].reshape(512, HID).T          # [HID, 512]
        wkT = Wk4[heads].reshape(512, HID).T
        wvT = Wv4[heads].reshape(512, HID).T          # [HID, 512]
        woT = Wo[:, hg * 512:(hg + 1) * 512].T        # [512, HID]
        m = {
            "hs_t": hs_tl[b],
            "wq_t": tile_qk(wqT),
            "wk_t": tile_qk(wkT),
            "wv_t": np.ascontiguousarray(
                wvT.reshape(16, 128, 512).transpose(1, 0, 2)),
            "wo_t": np.ascontiguousarray(
                woT.reshape(4, 128, 16, 128).transpose(1, 0, 2, 3)),
            "cosT": cosT,
            "ssinT": ssinT,
        }
        if nm:
            m["maskt"] = maskt
        in_maps.append(m)

    trace = bool(os.environ.get("CC_BASS_TRACE"))
    res = run_bass_kernel_spmd(nc, in_maps, core_ids=list(range(NCORES)), trace=trace)
    LAST_EXEC_NS = res.exec_time_ns

    out = np.empty((B, S, S), dtype=np.float32)
    for b in range(B):
        acc = res.results[b * HPC]["oT_t"].astype(np.float32)
        for hg in range(1, HPC):
            acc = acc + res.results[b * HPC + hg]["oT_t"]
        # [qc, iblk, p, t] -> [iblk*128+p, qc*512+t] = oT_full, out = oT_full.T
        o_full = acc.transpose(1, 2, 0, 3).reshape(S, S)
        out[b] = o_full.T
    return out
